# revision 1
# baseline (speedup 1.0000x reference)
"""Trainium2 Bass kernel for nn_Encoder (R-GCN style message passing).

Math (faithful to the reference, including its s-major/f-major index mismatch):
    supports_ = concat_s(A[s] @ features)            # [N, S*F], cols k=s*F+f
    Vmat      = (W_comp @ W.transpose(1,0,2)).reshape(S*F, E)   # rows k=f*S+s
    out       = supports_ @ Vmat

Rewritten as one big contraction:
    Q_s[f, e]  = Vmat[s*F + f, e]        (contiguous 32-row block of Vmat)
    H_s        = features @ Q_s          # [N, E]  (tiny)
    out        = sum_s A[s] @ H_s
               = Hcat.T-contract over (s, m):  out.T = Hcat.T @ Acat
    where Acat[(s,m), n] = A[s, n, m]  (host-transposed shard)
          Hcat[(s,m), e] = H_s[m, e]

Sharding: node dim N split across 8 cores (1024 rows each). Each core
streams its 128 MiB A-shard through the PE as the moving operand with
H-chunks as 128x32 stationary weights, accumulating out.T [32, 1024] in
PSUM. Host does layout-only transforms (transpose/replicate/shard) and
the final gather+transpose; all arithmetic runs on device.
"""

import os
import numpy as np

import concourse.bass as bass
import concourse.mybir as mybir
from concourse import bacc, bass_utils
from concourse.tile import TileContext
from concourse.tile_rust import add_dep_helper

S, N, F, E = 4, 8192, 32, 32
P = 128
N_CORES = 8
NS = N // N_CORES          # 1024 node rows per core
KTOT = S * N               # 32768 contraction rows
NCHUNK = KTOT // P         # 256 K-chunks of 128
JPB = int(os.environ.get("KJPB", "4"))   # K-chunks per DMA block
NBLK = NCHUNK // JPB       # DMA blocks
MB = N // (P * JPB)        # DMA blocks per relation
MCH = N // P               # 64 m-chunks per relation

# Matmul dtype for the big streaming matmul ('f32' | 'f32r' | 'fp16' | 'bf16').
# fp16 halves the HBM traffic for A (the sole large operand) and runs the PE
# at full rate; measured scaled absmax error ~3e-4 vs the fp32 reference.
MAIN_DT = os.environ.get("KDT", "fp16")

_DT_MAP = {
    "f32": (mybir.dt.float32, np.float32),
    "f32r": (mybir.dt.float32r, np.float32),
    "fp16": (mybir.dt.float16, np.float16),
}


def _np_dt(key):
    if key == "bf16":
        import ml_dtypes
        return ml_dtypes.bfloat16
    return _DT_MAP[key][1]


def _build(dt_key):
    """Build + finalize the per-core Bass program (same program on all cores)."""
    if dt_key == "bf16":
        dt_main = mybir.dt.bfloat16
    else:
        dt_main = _DT_MAP[dt_key][0]
    f32 = mybir.dt.float32
    f32r = mybir.dt.float32r
    # H-phase dtype: match main dtype for 2-byte modes (hcat is quantized to
    # it anyway; enables FWL fast weight loads), f32r otherwise.
    dt_h = f32r
    defbufs = (6 if dt_key in ("fp16", "bf16") else 3) * 8 // JPB
    abufs = int(os.environ.get("KABUFS", str(defbufs)))

    nc = bacc.Bacc("TRN2")
    atc = nc.dram_tensor("atc", [KTOT, NS], dt_main, kind="ExternalInput")
    featT = nc.dram_tensor("featT", [F, N], dt_h, kind="ExternalInput")
    # per-relation expanded basis weights, all at base partition 0:
    # wmat[f, s*64 + b*32 + e] = W[b, (s*32+f)//4, e] replicated per Vmat row
    # wcs[f, s*2 + b] = W_comp[(s*32+f)%4, b]
    wmat = nc.dram_tensor("wmat", [F, S * 2 * E], f32, kind="ExternalInput")
    wcs = nc.dram_tensor("wcs", [F, S * 2], f32, kind="ExternalInput")
    outT = nc.dram_tensor("outT", [E, NS], f32, kind="ExternalOutput")

    # Contraction rows permuted so partition p's block data is one contiguous
    # run: row k = b*(P*JPB) + p*JPB + j  (16-32 KB per partition per DMA).
    atc_r = atc.rearrange("(b p j) n -> b p (j n)", p=P, j=JPB)

    with TileContext(nc) as tc:
        with (
            tc.tile_pool(name="consts", bufs=1) as consts,
            tc.tile_pool(name="hcatp", bufs=1) as hcatp,
            tc.tile_pool(name="abuf", bufs=abufs) as apool,
            tc.tile_pool(name="hps", bufs=4, space="PSUM") as hps,
            tc.tile_pool(name="ops", bufs=1, space="PSUM") as opsum,
            tc.tile_pool(name="osb", bufs=1) as osb,
        ):
            # A-block loads alternate between the two independent HWDGE rings
            # (SP/sync and ACT/scalar) to double descriptor-issue throughput.
            def a_dma(b, ab):
                eng = nc.sync if b % 2 == 0 else nc.scalar
                eng.dma_start(ab, atc_r[b])

            # ---- kick off the first A-block loads before anything else ----
            pre = {}
            for b in range(min(4, NBLK)):
                ab = apool.tile([P, JPB * NS], dt_main)
                a_dma(b, ab)
                pre[b] = ab

            # ---- constants ----
            ft = consts.tile([F, N], dt_h)
            nc.sync.dma_start(ft, featT[:, :])
            wm = consts.tile([F, S * 2 * E], f32)
            nc.sync.dma_start(wm, wmat[:, :])
            wc = consts.tile([F, S * 2], f32)
            nc.sync.dma_start(wc, wcs[:, :])

            # ---- Q_s [32, 32] per relation: Q_s = wc0*W0blk + wc1*W1blk
            tmp = consts.tile([F, E], f32)
            qs = []
            for s in range(S):
                q = consts.tile([F, E], f32, tag=f"q{s}")
                nc.vector.tensor_scalar_mul(
                    tmp, wm[:, s * 64 : s * 64 + E], wc[:, 2 * s : 2 * s + 1]
                )
                nc.vector.tensor_scalar_mul(
                    q, wm[:, s * 64 + E : (s + 1) * 64], wc[:, 2 * s + 1 : 2 * s + 2]
                )
                nc.vector.tensor_add(q, q, tmp)
                qr = consts.tile([F, E], dt_h, tag=f"qr{s}")
                nc.any.tensor_copy(qr, q)
                qs.append(qr)

            # ---- Hcat [128, NCHUNK*E]: chunk c (= s*MCH + mc) at cols c*E:(c+1)*E,
            #      Hcat_chunk[p, e] = sum_f featT[f, mc*P+p] * Q_s[f, e]
            hcat = hcatp.tile([P, NCHUNK * E], dt_main)

            def emit_h_block(bb, after=None):
                # all JPB chunks of block bb packed into one PSUM tile, one copy
                # block bb covers rows k = bb*(P*JPB) + p*JPB + j -> s = bb // MB,
                # m = (bb % MB)*P*JPB + p*JPB + j; ft is host-permuted to
                # [f, (g, j, p)] so the weight slice is contiguous.
                # `after` throttles scheduler run-ahead: without it the Tile
                # scheduler clusters all H matmuls, starving the A-block DMAs
                # of buffer slots mid-kernel.
                s, g = divmod(bb, MB)
                hp = hps.tile([P, JPB * E], f32)
                for j in range(JPB):
                    mm = nc.tensor.matmul(
                        hp[:, j * E : (j + 1) * E],
                        ft[:, (g * JPB + j) * P : (g * JPB + j + 1) * P],
                        qs[s],
                        start=True,
                        stop=True,
                    )
                    if after is not None:
                        add_dep_helper(
                            mm.ins, after.ins, sync=False,
                            reason="throttle H run-ahead",
                        )
                nc.any.tensor_copy(
                    hcat[:, bb * JPB * E : (bb + 1) * JPB * E], hp
                )

            # ---- main streaming matmul: out.T += Hcat_chunk.T @ A_block
            ps0 = opsum.tile([E, 512], f32)
            ps1 = opsum.tile([E, 512], f32)

            emit_h_block(0)
            mm_hist = []
            for b in range(NBLK):
                if b in pre:
                    ab = pre.pop(b)
                else:
                    ab = apool.tile([P, JPB * NS], dt_main)
                    a_dma(b, ab)
                if b + 1 < NBLK:
                    # anchor two blocks back: H(b+1) may overlap main(b-1) and
                    # main(b), so the H->hcat-copy->main-MM chain never sits on
                    # the PE critical path, while run-ahead stays bounded.
                    anchor = mm_hist[-2] if len(mm_hist) >= 2 else None
                    emit_h_block(b + 1, after=anchor)
                for j in range(JPB):
                    c = b * JPB + j
                    hc = hcat[:, c * E : (c + 1) * E]
                    first = c == 0
                    last = c == NCHUNK - 1
                    nc.tensor.matmul(
                        ps0, hc, ab[:, j * NS : j * NS + 512],
                        start=first, stop=last, skip_group_check=True,
                    )
                    mm = nc.tensor.matmul(
                        ps1, hc, ab[:, j * NS + 512 : (j + 1) * NS],
                        start=first, stop=last, skip_group_check=True,
                    )
                mm_hist.append(mm)

            # split output halves across engines + both HWDGE rings so the
            # ps0 half's copy+store overlaps the ps1 half's
            ot0 = osb.tile([E, 512], f32, tag="ot0")
            ot1 = osb.tile([E, 512], f32, tag="ot1")
            nc.scalar.copy(ot0, ps0)
            nc.vector.tensor_copy(ot1, ps1)
            nc.sync.dma_start(outT[:, 0:512], ot0)
            nc.scalar.dma_start(outT[:, 512:NS], ot1)

    nc.finalize()
    return nc


_built_cache = {}


def _get_nc(dt_key):
    if dt_key not in _built_cache:
        _built_cache[dt_key] = _build(dt_key)
    return _built_cache[dt_key]


def _shard_inputs(features, A, W, W_comp, dt_key):
    np_main = _np_dt(dt_key)
    features = np.asarray(features, dtype=np.float32)
    A = np.asarray(A, dtype=np.float32)
    W = np.asarray(W, dtype=np.float32)
    W_comp = np.asarray(W_comp, dtype=np.float32)

    # featT columns ordered (g, j, p) to match the permuted contraction rows
    featT = np.ascontiguousarray(
        features.reshape(MB, P, JPB, F).transpose(3, 0, 2, 1).reshape(F, N)
    ).astype(np.float32)
    wmat_full = np.concatenate(
        [np.repeat(W[0], S, axis=0), np.repeat(W[1], S, axis=0)], axis=1
    ).astype(np.float32)                                          # [128, 2E], row k
    wcs_full = np.stack(
        [np.tile(W_comp[:, 0], F), np.tile(W_comp[:, 1], F)], axis=1
    ).astype(np.float32)                                          # [128, 2]
    # regroup rows k = s*32+f into per-s column blocks at partitions f=0..31
    wmat = np.ascontiguousarray(
        wmat_full.reshape(S, F, 2 * E).transpose(1, 0, 2).reshape(F, S * 2 * E)
    )
    wcs = np.ascontiguousarray(
        wcs_full.reshape(S, F, 2).transpose(1, 0, 2).reshape(F, S * 2)
    )

    in_maps = []
    for c in range(N_CORES):
        a_sh = A[:, c * NS : (c + 1) * NS, :]                     # [S, NS, M]
        atc = np.ascontiguousarray(a_sh.transpose(0, 2, 1)).reshape(KTOT, NS)
        in_maps.append(
            {
                "atc": atc.astype(np_main),
                "featT": featT,
                "wmat": wmat,
                "wcs": wcs,
            }
        )
    return in_maps


def _run(features, A, W, W_comp, dt_key=None, trace=False):
    dt_key = dt_key or MAIN_DT
    nc = _get_nc(dt_key)
    in_maps = _shard_inputs(features, A, W, W_comp, dt_key)
    res = bass_utils.run_bass_kernel_spmd(
        nc, in_maps, core_ids=list(range(N_CORES)), trace=trace
    )
    out = np.concatenate(
        [res.results[c]["outT"].T for c in range(N_CORES)], axis=0
    ).astype(np.float32)
    return out, res


def kernel(features, A, W, W_comp):
    try:
        out, _ = _run(features, A, W, W_comp)
    except Exception:
        # Rare transient device-unrecoverable flakes: reset jax backends and
        # retry once with a freshly built program.
        import jax
        try:
            jax.clear_caches()
            jax.extend.backend.clear_backends()
        except Exception:
            pass
        _built_cache.clear()
        out, _ = _run(features, A, W, W_comp)
    return out



# revision 2
# speedup vs baseline: 1.3494x; 1.3494x over previous
"""Trainium2 Bass kernel for nn_Encoder (R-GCN style message passing).

Math (faithful to the reference, including its s-major/f-major index mismatch):
    supports_ = concat_s(A[s] @ features)            # [N, S*F], cols k=s*F+f
    Vmat      = (W_comp @ W.transpose(1,0,2)).reshape(S*F, E)   # rows k=f*S+s
    out       = supports_ @ Vmat
Equivalent per-relation form with Q_eff[s][f, e] = Vmat[s*F + f, e]:
    H_s = features @ Q_eff[s]          # [N, E]  (tiny)
    out = sum_s A[s] @ H_s

Sharding: output rows n split across 8 cores (NS=1024 each); each core
contracts its A[:, n_shard, :] against replicated features/W/W_comp.

Device kernel (per core):
  - A-shard streamed from HBM as float8_e3m4 (1 B/elem -> ~33.5 MB), the
    sole large operand; this halves DMA vs fp16 and is the memory roofline.
  - Contraction rows k ordered m-major: chunk c = mc*S + s, so the H tile
    for m-chunk mc is produced just-in-time by ONE [32,128]x[32,128] PE
    matmul covering all 4 relations (stationary ft chunk, moving qcat).
  - Main loop: out.T[E, NS] += Hc.T @ A_chunk with Hc [128,32] fp16
    stationary and A [128,512] fp8e3 moving (mixed-dtype matmul; PE
    streams 1 col/cycle -> ~110 us, just above the ~95 us DMA floor).
Host does layout-only transforms + the e3m4/fp16 casts; final gather is
a concat of per-core outT.T blocks.
"""

import os
import numpy as np
import ml_dtypes

import concourse.bass as bass
import concourse.mybir as mybir
from concourse import bacc, bass_utils
from concourse.tile import TileContext
from concourse.tile_rust import add_dep_helper

S, N, F, E = 4, 8192, 32, 32
P = 128
N_CORES = 8
NS = N // N_CORES          # 1024 output rows per core
KTOT = S * N               # 32768 contraction rows
NCHUNK = KTOT // P         # 256 K-chunks of 128
JPB = int(os.environ.get("KJPB", "8"))   # K-chunks per DMA block (mult of S)
NBLK = NCHUNK // JPB       # DMA blocks
HPB = JPB // S             # H (m-chunk) tiles per block
NMC = N // P               # 64 m-chunks

# A dtype ('f8e3' | 'fp16') and H/hcat dtype ('fp16' | 'bf16' | 'f8e3').
ADT = os.environ.get("KDT", "f8e3")
HDT = os.environ.get("KHD", "fp16")

_DT_MAP = {
    "f8e3": (mybir.dt.float8e3, ml_dtypes.float8_e3m4),
    "fp16": (mybir.dt.float16, np.float16),
    "bf16": (mybir.dt.bfloat16, ml_dtypes.bfloat16),
}


def _build(adt_key, hdt_key):
    """Build + finalize the per-core Bass program (same program on all cores)."""
    dt_a, _ = _DT_MAP[adt_key]
    dt_h, _ = _DT_MAP[hdt_key]
    f32 = mybir.dt.float32
    abufs = int(os.environ.get("KABUFS", "6"))

    nc = bacc.Bacc("TRN2")
    atc = nc.dram_tensor("atc", [KTOT, NS], dt_a, kind="ExternalInput")
    featT = nc.dram_tensor("featT", [F, N], mybir.dt.float16, kind="ExternalInput")
    # wpk cols: [wb0 | wb1 | wc0 | wc1], each [32, 128] with
    # wb_b[f, s*E+e] = W[b, (s*F+f)//S, e], wc_b[f, s*E+e] = W_comp[(s*F+f)%S, b]
    wpk = nc.dram_tensor("wpk", [F, 4 * S * E], f32, kind="ExternalInput")
    outT = nc.dram_tensor("outT", [E, NS], f32, kind="ExternalOutput")

    # Contraction rows permuted so partition p's block data is one contiguous
    # run of JPB*NS bytes: row k = b*(P*JPB) + p*JPB + j.
    atc_r = atc.rearrange("(b p j) n -> b p (j n)", p=P, j=JPB)

    with TileContext(nc) as tc:
        with (
            tc.tile_pool(name="consts", bufs=1) as consts,
            tc.tile_pool(name="hcatp", bufs=8) as hcatp,
            tc.tile_pool(name="abuf", bufs=abufs) as apool,
            tc.tile_pool(name="hps", bufs=4, space="PSUM") as hps,
            tc.tile_pool(name="ops", bufs=1, space="PSUM") as opsum,
            tc.tile_pool(name="osb", bufs=1) as osb,
        ):
            # ---- constants first (tiny, on sync ring) ----
            ft = consts.tile([F, N], mybir.dt.float16)
            nc.sync.dma_start(ft, featT[:, :])
            wp = consts.tile([F, 4 * S * E], f32)
            nc.sync.dma_start(wp, wpk[:, :])

            # A-block loads alternate between the two independent HWDGE rings
            # (scalar/ACT and sync/SP) to double descriptor-issue throughput.
            def a_dma(b, ab):
                eng = nc.scalar if b % 2 == 0 else nc.sync
                eng.dma_start(ab, atc_r[b])

            pre = {}
            for b in range(min(4, NBLK)):
                ab = apool.tile([P, JPB * NS], dt_a)
                a_dma(b, ab)
                pre[b] = ab

            # ---- qcat [32, 128]: qcat[f, s*E+e] = V[s*F+f, e] ----
            qtmp = consts.tile([F, S * E], f32)
            qf = consts.tile([F, S * E], f32)
            nc.vector.tensor_mul(qtmp, wp[:, 0 : S * E], wp[:, 2 * S * E : 3 * S * E])
            nc.vector.tensor_mul(qf, wp[:, S * E : 2 * S * E], wp[:, 3 * S * E :])
            nc.vector.tensor_add(qf, qf, qtmp)
            qcat = consts.tile([F, S * E], mybir.dt.float16)
            nc.vector.tensor_copy(qcat, qf)

            # ---- main loop: H tiles just-in-time + streaming A matmul ----
            ps0 = opsum.tile([E, 512], f32)
            ps1 = opsum.tile([E, 512], f32)

            hct = {}

            def emit_h(mc, after=None):
                # H4 tile [128 m, (s,e)]: all 4 relations for m-chunk mc in one
                # matmul: ft[:, mc*P:(mc+1)*P].T @ qcat. `after` throttles
                # scheduler run-ahead so H work tracks the main loop.
                hp = hps.tile([P, S * E], f32)
                mm = nc.tensor.matmul(
                    hp, ft[:, mc * P : (mc + 1) * P], qcat, start=True, stop=True
                )
                if after is not None:
                    add_dep_helper(mm.ins, after.ins, sync=False,
                                   reason="throttle H run-ahead")
                t = hcatp.tile([P, S * E], dt_h)
                nc.any.tensor_copy(t, hp)
                hct[mc] = t

            mm_hist = []
            for b in range(NBLK):
                if b in pre:
                    ab = pre.pop(b)
                else:
                    ab = apool.tile([P, JPB * NS], dt_a)
                    a_dma(b, ab)
                anchor = mm_hist[-2] if len(mm_hist) >= 2 else None
                for h in range(HPB):
                    emit_h(b * HPB + h, after=anchor)
                for j in range(JPB):
                    c = b * JPB + j
                    s, h = j % S, j // S
                    hc = hct[b * HPB + h][:, s * E : (s + 1) * E]
                    first = c == 0
                    last = c == NCHUNK - 1
                    nc.tensor.matmul(
                        ps0, hc, ab[:, j * NS : j * NS + 512],
                        start=first, stop=last, skip_group_check=True,
                    )
                    mm = nc.tensor.matmul(
                        ps1, hc, ab[:, j * NS + 512 : (j + 1) * NS],
                        start=first, stop=last, skip_group_check=True,
                    )
                mm_hist.append(mm)
                for h in range(HPB):
                    del hct[b * HPB + h]

            # split output halves across engines + both HWDGE rings
            ot0 = osb.tile([E, 512], f32, tag="ot0")
            ot1 = osb.tile([E, 512], f32, tag="ot1")
            nc.scalar.copy(ot0, ps0)
            nc.vector.tensor_copy(ot1, ps1)
            nc.sync.dma_start(outT[:, 0:512], ot0)
            nc.scalar.dma_start(outT[:, 512:NS], ot1)

    nc.finalize()
    return nc


_built_cache = {}


def _get_nc(adt_key, hdt_key):
    key = (adt_key, hdt_key)
    if key not in _built_cache:
        _built_cache[key] = _build(adt_key, hdt_key)
    return _built_cache[key]


def _shard_inputs(features, A, W, W_comp, adt_key):
    _, np_a = _DT_MAP[adt_key]
    features = np.asarray(features, dtype=np.float32)
    A = np.asarray(A, dtype=np.float32)
    W = np.asarray(W, dtype=np.float32)
    W_comp = np.asarray(W_comp, dtype=np.float32)

    featT = np.ascontiguousarray(features.T).astype(np.float16)

    # wpk blocks, replicating the reference's (s*F+f) <-> (f*S+s) pairing
    kmat = np.arange(S * E).reshape(S, E)          # kmat[s, f] = s*F + f
    wb = W[:, kmat // S, :]                        # [2, s, f, e]
    wc = W_comp[kmat % S, :]                       # [s, f, b]
    blocks = [
        wb[0].transpose(1, 0, 2).reshape(F, S * E),
        wb[1].transpose(1, 0, 2).reshape(F, S * E),
        np.repeat(wc[:, :, 0].T[:, :, None], E, axis=2).reshape(F, S * E),
        np.repeat(wc[:, :, 1].T[:, :, None], E, axis=2).reshape(F, S * E),
    ]
    wpk = np.ascontiguousarray(np.concatenate(blocks, axis=1)).astype(np.float32)

    # One cast of the full A, then per-core byte gathers.
    A8 = A.astype(np_a)                            # [S, n, m]
    AT = np.ascontiguousarray(A8.transpose(0, 2, 1))   # [S, m, n]

    in_maps = []
    for c in range(N_CORES):
        blk = AT[:, :, c * NS : (c + 1) * NS]      # [S, N(m), NS]
        # rows k = b*(P*JPB) + p*JPB + (h*S + s), m = (b*HPB + h)*P + p
        at2 = blk.reshape(S, NBLK, HPB, P, NS).transpose(1, 3, 2, 0, 4)
        atc = np.ascontiguousarray(at2).reshape(KTOT, NS)
        in_maps.append({"atc": atc, "featT": featT, "wpk": wpk})
    return in_maps


def _run(features, A, W, W_comp, dt_key=None, trace=False):
    adt_key = dt_key or ADT
    nc = _get_nc(adt_key, HDT)
    in_maps = _shard_inputs(features, A, W, W_comp, adt_key)
    res = bass_utils.run_bass_kernel_spmd(
        nc, in_maps, core_ids=list(range(N_CORES)), trace=trace
    )
    out = np.concatenate(
        [res.results[c]["outT"].T for c in range(N_CORES)], axis=0
    ).astype(np.float32)
    return out, res


def kernel(features, A, W, W_comp):
    try:
        out, _ = _run(features, A, W, W_comp)
    except Exception:
        # Rare transient device-unrecoverable flakes: reset jax backends and
        # retry once with a freshly built program.
        import jax
        try:
            jax.clear_caches()
            jax.extend.backend.clear_backends()
        except Exception:
            pass
        _built_cache.clear()
        out, _ = _run(features, A, W, W_comp)
    return out


# revision 9
# speedup vs baseline: 1.4122x; 1.0466x over previous
"""Trainium2 Bass kernel for nn_Encoder (R-GCN style message passing).

Math (faithful to the reference, including its s-major/f-major index mismatch):
    supports_ = concat_s(A[s] @ features)            # [N, S*F], cols k=s*F+f
    Vmat      = (W_comp @ W.transpose(1,0,2)).reshape(S*F, E)   # rows k=f*S+s
    out       = supports_ @ Vmat
Equivalent per-relation form with Q_eff[s][f, e] = Vmat[s*F + f, e]:
    H_s = features @ Q_eff[s]          # [N, E]  (tiny)
    out = sum_s A[s] @ H_s

Sharding: output rows n split across 8 cores (NS=1024 each); each core
contracts its A[:, n_shard, :] against replicated features/W/W_comp.

Device kernel (per core):
  - A-shard streamed from HBM as float8_e3m4 (1 B/elem -> ~33.5 MB), the
    sole large operand; this halves DMA vs fp16 and is the memory roofline.
  - Contraction rows k ordered m-major: chunk c = mc*S + s, so the H tile
    for m-chunk mc is produced just-in-time by ONE [32,128]x[32,128] PE
    matmul covering all 4 relations (stationary ft chunk, moving qcat).
  - Main loop: out.T[E, NS] += Hc.T @ A_chunk with Hc [128,32] fp16
    stationary and A [128,512] fp8e3 moving (mixed-dtype matmul; PE
    streams 1 col/cycle -> ~110 us, just above the ~95 us DMA floor).
Host does layout-only transforms + the e3m4/fp16 casts; final gather is
a concat of per-core outT.T blocks.
"""

import os
import numpy as np
import ml_dtypes

import concourse.bass as bass
import concourse.mybir as mybir
from concourse import bacc, bass_utils
from concourse.tile import TileContext
from concourse.tile_rust import add_dep_helper

S, N, F, E = 4, 8192, 32, 32
P = 128
N_CORES = 8
NS = N // N_CORES          # 1024 output rows per core
KTOT = S * N               # 32768 contraction rows
NCHUNK = KTOT // P         # 256 K-chunks of 128
JPB = int(os.environ.get("KJPB", "8"))   # K-chunks per DMA block (mult of S)
NBLK = NCHUNK // JPB       # DMA blocks
HPB = JPB // S             # H (m-chunk) tiles per block
NMC = N // P               # 64 m-chunks

# A dtype ('f8e3' | 'fp16') and H/hcat dtype ('fp16' | 'bf16' | 'f8e3').
ADT = os.environ.get("KDT", "f8e3")
HDT = os.environ.get("KHD", "fp16")
# Row-tiled H phase: pack 4 K=32 H-matmuls into the PE's 4 row groups via
# tile_position so they run concurrently (~3x on K=32 matmuls).
HTILE = os.environ.get("KHTILE", "1") == "1"
GQ = 4                     # m-chunks per row-tiled H group
NGRP = NMC // GQ           # 16 H groups

_DT_MAP = {
    "f8e3": (mybir.dt.float8e3, ml_dtypes.float8_e3m4),
    "fp16": (mybir.dt.float16, np.float16),
    "bf16": (mybir.dt.bfloat16, ml_dtypes.bfloat16),
}


def _build(adt_key, hdt_key):
    """Build + finalize the per-core Bass program (same program on all cores)."""
    dt_a, _ = _DT_MAP[adt_key]
    dt_h, _ = _DT_MAP[hdt_key]
    f32 = mybir.dt.float32
    abufs = int(os.environ.get("KABUFS", "6"))

    nc = bacc.Bacc("TRN2")
    atc = nc.dram_tensor("atc", [KTOT, NS], dt_a, kind="ExternalInput")
    # HTILE: featT[i*F + f, g*P + c] = features[(g*GQ+i)*P + c, f] -- the 4
    # row-group copies stacked on partitions. Else plain features.T [F, N].
    FTP = GQ * F if HTILE else F           # featT partitions
    featT = nc.dram_tensor(
        "featT", [FTP, N // (GQ if HTILE else 1)], mybir.dt.float16,
        kind="ExternalInput",
    )
    # wpk cols: [wb0 | wb1 | wc0 | wc1], each [F, 128] with
    # wb_b[f, s*E+e] = W[b, (s*F+f)//S, e], wc_b[f, s*E+e] = W_comp[(s*F+f)%S, b]
    # (HTILE: rows tiled 4x on partitions to feed the 4 row groups.)
    wpk = nc.dram_tensor("wpk", [FTP, 4 * S * E], f32, kind="ExternalInput")
    outT = nc.dram_tensor("outT", [E, NS], f32, kind="ExternalOutput")

    # Contraction rows permuted so partition p's block data is one contiguous
    # run of JPB*NS bytes: row k = b*(P*JPB) + p*JPB + j.
    atc_r = atc.rearrange("(b p j) n -> b p (j n)", p=P, j=JPB)

    with TileContext(nc) as tc:
        with (
            tc.tile_pool(name="consts", bufs=1) as consts,
            tc.tile_pool(name="hcatp", bufs=12) as hcatp,
            tc.tile_pool(name="abuf", bufs=abufs) as apool,
            tc.tile_pool(name="hps", bufs=6, space="PSUM") as hps,
            tc.tile_pool(name="ops", bufs=1, space="PSUM") as opsum,
            tc.tile_pool(name="osb", bufs=1) as osb,
        ):
            # ---- constants first (tiny, on sync ring) ----
            ft = consts.tile([F, N], mybir.dt.float16)
            nc.sync.dma_start(ft, featT[:, :])
            wp = consts.tile([F, 4 * S * E], f32)
            nc.sync.dma_start(wp, wpk[:, :])

            # Each A block is split in half across the two HWDGE rings
            # (SP/sync and ACT/scalar) so both rings stream every block
            # concurrently. The sync/scalar queues carry ONLY dma_starts:
            # a dma_start parked on its abuf WAR semaphore would stall any
            # copy queued behind it (FIFO), gating the PE.
            HCOL = JPB * NS // 2

            def a_dma(b, ab):
                nc.scalar.dma_start(ab[:, :HCOL], atc_r[b][:, :HCOL])
                nc.sync.dma_start(ab[:, HCOL:], atc_r[b][:, HCOL:])

            pre = {}
            for b in range(min(4, NBLK)):
                ab = apool.tile([P, JPB * NS], dt_a)
                a_dma(b, ab)
                pre[b] = ab

            # ---- qcat [32, 128]: qcat[f, s*E+e] = V[s*F+f, e] ----
            qtmp = consts.tile([F, S * E], f32)
            qf = consts.tile([F, S * E], f32)
            nc.vector.tensor_mul(qtmp, wp[:, 0 : S * E], wp[:, 2 * S * E : 3 * S * E])
            nc.vector.tensor_mul(qf, wp[:, S * E : 2 * S * E], wp[:, 3 * S * E :])
            nc.vector.tensor_add(qf, qf, qtmp)
            qcat = consts.tile([F, S * E], mybir.dt.float16)
            nc.vector.tensor_copy(qcat, qf)

            # ---- main loop: H tiles just-in-time + streaming A matmul ----
            ps0 = opsum.tile([E, 512], f32)
            ps1 = opsum.tile([E, 512], f32)

            hct = {}
            throttle = os.environ.get("KTHROTTLE", "0") == "1"

            def emit_h(mc, after=None):
                # H4 tile [128 m, (s,e)]: all 4 relations for m-chunk mc in one
                # matmul: ft[:, mc*P:(mc+1)*P].T @ qcat. Copies go on vector/
                # gpsimd (NOT sync/scalar, whose queues are DMA-only).
                hp = hps.tile([P, S * E], f32)
                mm = nc.tensor.matmul(
                    hp, ft[:, mc * P : (mc + 1) * P], qcat, start=True, stop=True
                )
                if after is not None:
                    add_dep_helper(mm.ins, after.ins, sync=False,
                                   reason="throttle H run-ahead")
                # GPSIMD cannot read PSUM; vector handles all PSUM->SBUF copies
                t = hcatp.tile([P, S * E], dt_h)
                nc.vector.tensor_copy(t, hp)
                hct[mc] = t

            mm_hist = []
            for b in range(NBLK):
                if b in pre:
                    ab = pre.pop(b)
                else:
                    ab = apool.tile([P, JPB * NS], dt_a)
                    a_dma(b, ab)
                anchor = (mm_hist[-2] if len(mm_hist) >= 2 else None) if throttle else None
                for h in range(HPB):
                    emit_h(b * HPB + h, after=anchor)
                for j in range(JPB):
                    c = b * JPB + j
                    s, h = j % S, j // S
                    hc = hct[b * HPB + h][:, s * E : (s + 1) * E]
                    first = c == 0
                    last = c == NCHUNK - 1
                    nc.tensor.matmul(
                        ps0, hc, ab[:, j * NS : j * NS + 512],
                        start=first, stop=last, skip_group_check=True,
                    )
                    mm = nc.tensor.matmul(
                        ps1, hc, ab[:, j * NS + 512 : (j + 1) * NS],
                        start=first, stop=last, skip_group_check=True,
                    )
                mm_hist.append(mm)
                for h in range(HPB):
                    del hct[b * HPB + h]

            # split output halves across engines + both HWDGE rings
            ot0 = osb.tile([E, 512], f32, tag="ot0")
            ot1 = osb.tile([E, 512], f32, tag="ot1")
            nc.vector.tensor_copy(ot0, ps0)
            nc.vector.tensor_copy(ot1, ps1)
            nc.sync.dma_start(outT[:, 0:512], ot0)
            nc.scalar.dma_start(outT[:, 512:NS], ot1)

    nc.finalize()
    return nc


_built_cache = {}


def _get_nc(adt_key, hdt_key):
    key = (adt_key, hdt_key)
    if key not in _built_cache:
        _built_cache[key] = _build(adt_key, hdt_key)
    return _built_cache[key]


def _shard_inputs(features, A, W, W_comp, adt_key):
    _, np_a = _DT_MAP[adt_key]
    features = np.asarray(features, dtype=np.float32)
    A = np.asarray(A, dtype=np.float32)
    W = np.asarray(W, dtype=np.float32)
    W_comp = np.asarray(W_comp, dtype=np.float32)

    featT = np.ascontiguousarray(features.T).astype(np.float16)

    # wpk blocks, replicating the reference's (s*F+f) <-> (f*S+s) pairing
    kmat = np.arange(S * E).reshape(S, E)          # kmat[s, f] = s*F + f
    wb = W[:, kmat // S, :]                        # [2, s, f, e]
    wc = W_comp[kmat % S, :]                       # [s, f, b]
    blocks = [
        wb[0].transpose(1, 0, 2).reshape(F, S * E),
        wb[1].transpose(1, 0, 2).reshape(F, S * E),
        np.repeat(wc[:, :, 0].T[:, :, None], E, axis=2).reshape(F, S * E),
        np.repeat(wc[:, :, 1].T[:, :, None], E, axis=2).reshape(F, S * E),
    ]
    wpk = np.ascontiguousarray(np.concatenate(blocks, axis=1)).astype(np.float32)

    # One cast of the full A, then per-core byte gathers.
    A8 = A.astype(np_a)                            # [S, n, m]
    AT = np.ascontiguousarray(A8.transpose(0, 2, 1))   # [S, m, n]

    in_maps = []
    for c in range(N_CORES):
        blk = AT[:, :, c * NS : (c + 1) * NS]      # [S, N(m), NS]
        # rows k = b*(P*JPB) + p*JPB + (h*S + s), m = (b*HPB + h)*P + p
        at2 = blk.reshape(S, NBLK, HPB, P, NS).transpose(1, 3, 2, 0, 4)
        atc = np.ascontiguousarray(at2).reshape(KTOT, NS)
        in_maps.append({"atc": atc, "featT": featT, "wpk": wpk})
    return in_maps


def _run(features, A, W, W_comp, dt_key=None, trace=False):
    adt_key = dt_key or ADT
    nc = _get_nc(adt_key, HDT)
    in_maps = _shard_inputs(features, A, W, W_comp, adt_key)
    res = bass_utils.run_bass_kernel_spmd(
        nc, in_maps, core_ids=list(range(N_CORES)), trace=trace
    )
    out = np.concatenate(
        [res.results[c]["outT"].T for c in range(N_CORES)], axis=0
    ).astype(np.float32)
    return out, res


def kernel(features, A, W, W_comp):
    try:
        out, _ = _run(features, A, W, W_comp)
    except Exception:
        # Rare transient device-unrecoverable flakes: reset jax backends and
        # retry once with a freshly built program.
        import jax
        try:
            jax.clear_caches()
            jax.extend.backend.clear_backends()
        except Exception:
            pass
        _built_cache.clear()
        out, _ = _run(features, A, W, W_comp)
    return out
